# revision 3
# baseline (speedup 1.0000x reference)
"""Trainium2 Bass kernel for nn_CrossAttention (B=4, LQ=4096, S=4096, D=512).

Sharding: data-parallel over (batch, query-half): core = 2*b + half.
Each core computes cross-attention for one batch element and 2048 query rows.
K/V projections are recomputed by both cores of a pair (cheap relative to
the attention matmuls).

All heavy matmuls run in bf16 with fp32 PSUM accumulation. Softmax skips the
max-subtraction (scores are bounded ~ +-30 here, exp stays in fp32 range) and
the normalization division is deferred to after the context matmul.
"""

import numpy as np

B, LQ, S = 4, 4096, 4096
D = 512          # SRC == TGT == 512
P = 128
LQH = LQ // 2    # 2048 query rows per core
DC = D // P      # 4 chunks of the feature dims
SC = S // P      # 32 s-chunks
IB = 512         # i-block (query) width processed per attention pass
NB = LQH // IB   # 4 blocks
KB = S // IB     # 8 s-blocks of 512 for the k projection

_CACHED = {}


def _build_program():
    import concourse.bass as bass
    import concourse.mybir as mybir
    import concourse.tile as tile
    from concourse import bacc
    from concourse.masks import make_identity
    from contextlib import ExitStack

    f32 = mybir.dt.float32
    bf16 = mybir.dt.bfloat16
    AF = mybir.ActivationFunctionType
    OP = mybir.AluOpType

    nc = bacc.Bacc("TRN2", target_bir_lowering=False, debug=False, num_devices=8)

    query = nc.dram_tensor("query", [LQH, D], f32, kind="ExternalInput").ap()
    target = nc.dram_tensor("target", [S, D], f32, kind="ExternalInput").ap()
    w_dram = {}
    b_dram = {}
    for nm in ("wq", "wk", "wv", "wo"):
        w_dram[nm] = nc.dram_tensor(nm, [D, D], f32, kind="ExternalInput").ap()
    for nm in ("bq", "bk", "bv", "bo"):
        b_dram[nm] = nc.dram_tensor(nm, [D], f32, kind="ExternalInput").ap()
    out_dram = nc.dram_tensor("out", [LQH, D], f32, kind="ExternalOutput").ap()

    with tile.TileContext(nc) as tc, ExitStack() as ctx:
        const = ctx.enter_context(tc.tile_pool(name="const", bufs=1))
        big = ctx.enter_context(tc.tile_pool(name="big", bufs=1))
        wstage = ctx.enter_context(tc.tile_pool(name="wstage", bufs=1))
        ld = ctx.enter_context(tc.tile_pool(name="ld", bufs=3))
        ptp = ctx.enter_context(tc.tile_pool(name="ptp", bufs=3))
        ctxp = ctx.enter_context(tc.tile_pool(name="ctxp", bufs=2))
        outp = ctx.enter_context(tc.tile_pool(name="outp", bufs=2))
        smallp = ctx.enter_context(tc.tile_pool(name="smallp", bufs=4))
        ps_small = ctx.enter_context(tc.tile_pool(name="ps_small", bufs=2, space="PSUM"))
        ps_acc = ctx.enter_context(tc.tile_pool(name="ps_acc", bufs=5, space="PSUM"))
        ps_rs = ctx.enter_context(tc.tile_pool(name="ps_rs", bufs=1, space="PSUM"))

        # ---- constants ----
        ident = const.tile([P, P], f32, tag="ident", name="ident")
        make_identity(nc, ident)

        ones_col = const.tile([P, 1], bf16, tag="ones_col", name="ones_col")
        nc.vector.memset(ones_col, 1.0)
        ones_row = const.tile([1, P], f32, tag="ones_row", name="ones_row")
        nc.vector.memset(ones_row, 1.0)

        # weights -> bf16 [P, DC, D] (contraction dim on partitions)
        w_bf = {}
        for nm in ("wq", "wk", "wv", "wo"):
            wf = wstage.tile([P, DC, D], f32, tag="w_stage", name=f"{nm}_f32")
            nc.sync.dma_start(wf, w_dram[nm].rearrange("(c p) n -> p c n", p=P))
            wb = const.tile([P, DC, D], bf16, tag=f"w_{nm}", name=f"{nm}_bf")
            nc.vector.tensor_copy(out=wb, in_=wf)
            w_bf[nm] = wb

        # per-partition bias columns for q/k (t on partitions)
        b_col = {}
        for nm in ("bq", "bk"):
            bc = const.tile([P, DC], f32, tag=f"col_{nm}", name=f"{nm}_col")
            nc.gpsimd.dma_start(out=bc, in_=b_dram[nm].rearrange("(c p) -> p c", p=P))
            b_col[nm] = bc
        # broadcast-replicated bias rows for v/o (feature on free dim)
        b_rep = {}
        for nm in ("bv", "bo"):
            br = const.tile([P, D], f32, tag=f"rep_{nm}", name=f"{nm}_rep")
            src = b_dram[nm]
            bcast = bass.AP(tensor=src.tensor, offset=src.offset,
                            ap=[[0, P]] + list(src.ap))
            nc.gpsimd.dma_start(out=br, in_=bcast)
            b_rep[nm] = br

        # ---- stage A: load + transpose inputs to bf16 [d-part, seq] ----
        tgtT = big.tile([P, DC, S], bf16, tag="tgtT", name="tgtT")
        qinT = big.tile([P, DC, LQH], bf16, tag="qinT", name="qinT")

        def load_transposed(src, n_chunks, dstT):
            for scc in range(n_chunks):
                chunk = ld.tile([P, D], f32, tag="in_chunk", name=f"chk_{dstT.name}_{scc}")
                nc.sync.dma_start(chunk, src[scc * P:(scc + 1) * P, :])
                ps = ps_small.tile([P, D], f32, tag="ps_small", name=f"pst_{dstT.name}_{scc}")
                for dc in range(DC):
                    nc.tensor.transpose(ps[:, dc * P:(dc + 1) * P],
                                        chunk[:, dc * P:(dc + 1) * P], ident)
                nc.scalar.activation(dstT[:, :, scc * P:(scc + 1) * P],
                                     ps.rearrange("p (c q) -> p c q", c=DC),
                                     AF.Copy)

        load_transposed(query, LQH // P, qinT)
        load_transposed(target, SC, tgtT)

        # ---- stage B: projections ----
        kT = big.tile([P, DC, S], bf16, tag="kT", name="kT")
        qT = big.tile([P, DC, LQH], bf16, tag="qT", name="qT")
        vv = big.tile([P, SC, D], bf16, tag="vv", name="vv")

        # qT[t, i] = sum_d wq[d, t] * qinT[d, i]  (+bq per-partition)
        for tcc in range(DC):
            for ibk in range(NB):
                ps = ps_small.tile([P, IB], f32, tag="ps_small", name=f"psq_{tcc}_{ibk}")
                for dc in range(DC):
                    nc.tensor.matmul(ps, w_bf["wq"][:, dc, tcc * P:(tcc + 1) * P],
                                     qinT[:, dc, ibk * IB:(ibk + 1) * IB],
                                     start=(dc == 0), stop=(dc == DC - 1))
                nc.vector.tensor_tensor(qT[:, tcc, ibk * IB:(ibk + 1) * IB], ps,
                                        b_col["bq"][:, tcc:tcc + 1].to_broadcast([P, IB]),
                                        OP.add)

        # kT[t, s] = sum_d wk[d, t] * tgtT[d, s]  (+bk per-partition)
        for tcc in range(DC):
            for sb in range(KB):
                ps = ps_small.tile([P, IB], f32, tag="ps_small", name=f"psk_{tcc}_{sb}")
                for dc in range(DC):
                    nc.tensor.matmul(ps, w_bf["wk"][:, dc, tcc * P:(tcc + 1) * P],
                                     tgtT[:, dc, sb * IB:(sb + 1) * IB],
                                     start=(dc == 0), stop=(dc == DC - 1))
                nc.vector.tensor_tensor(kT[:, tcc, sb * IB:(sb + 1) * IB], ps,
                                        b_col["bk"][:, tcc:tcc + 1].to_broadcast([P, IB]),
                                        OP.add)

        # v[s, d'] = sum_d tgtT[d, s] * wv[d, d']  (+bv broadcast row)
        for scc in range(SC):
            ps = ps_small.tile([P, D], f32, tag="ps_small", name=f"psv_{scc}")
            for dc in range(DC):
                nc.tensor.matmul(ps, tgtT[:, dc, scc * P:(scc + 1) * P],
                                 w_bf["wv"][:, dc, :],
                                 start=(dc == 0), stop=(dc == DC - 1))
            nc.vector.tensor_tensor(vv[:, scc, :], ps, b_rep["bv"], OP.add)

        # ---- stage C: attention + output projection, per 512-wide i block ----
        for ib in range(NB):
            isl = slice(ib * IB, (ib + 1) * IB)
            ctx_ps = [ps_acc.tile([P, IB], f32, tag="ps_acc", name=f"ctx_{ib}_{d}")
                      for d in range(DC)]
            rs_ps = ps_rs.tile([1, IB], f32, tag="rs", name=f"rs_{ib}")

            for scc in range(SC):
                pt_ps = ps_small.tile([P, IB], f32, tag="ps_small", name=f"pt_{ib}_{scc}")
                for tcc in range(DC):
                    nc.tensor.matmul(pt_ps, kT[:, tcc, scc * P:(scc + 1) * P],
                                     qT[:, tcc, isl],
                                     start=(tcc == 0), stop=(tcc == DC - 1))
                pt_exp = ptp.tile([P, IB], bf16, tag="pt_exp", name=f"pte_{ib}_{scc}")
                nc.scalar.activation(pt_exp, pt_ps, AF.Exp)
                for dpc in range(DC):
                    nc.tensor.matmul(ctx_ps[dpc], vv[:, scc, dpc * P:(dpc + 1) * P],
                                     pt_exp, start=(scc == 0), stop=(scc == SC - 1))
                nc.tensor.matmul(rs_ps, ones_col, pt_exp,
                                 start=(scc == 0), stop=(scc == SC - 1))

            recip = smallp.tile([1, IB], f32, tag="recip", name=f"recip_{ib}")
            nc.vector.reciprocal(recip, rs_ps)
            rb_ps = ps_acc.tile([P, IB], f32, tag="ps_acc", name=f"rb_{ib}")
            nc.tensor.matmul(rb_ps, ones_row, recip, start=True, stop=True)
            rb_sb = smallp.tile([P, IB], f32, tag="rb_sb", name=f"rb_sb_{ib}")
            nc.scalar.activation(rb_sb, rb_ps, AF.Copy)

            ctxT = ctxp.tile([P, DC, IB], bf16, tag="ctxT", name=f"ctxT_{ib}")
            for dpc in range(DC):
                nc.vector.tensor_tensor(ctxT[:, dpc, :], ctx_ps[dpc], rb_sb, OP.mult)

            for ic in range(DC):
                op_ps = ps_acc.tile([P, D], f32, tag="ps_acc", name=f"op_{ib}_{ic}")
                for dpc in range(DC):
                    nc.tensor.matmul(op_ps, ctxT[:, dpc, ic * P:(ic + 1) * P],
                                     w_bf["wo"][:, dpc, :],
                                     start=(dpc == 0), stop=(dpc == DC - 1))
                ot = outp.tile([P, D], f32, tag="out_t", name=f"ot_{ib}_{ic}")
                nc.vector.tensor_tensor(ot, op_ps, b_rep["bo"], OP.add)
                nc.sync.dma_start(out_dram[ib * IB + ic * P: ib * IB + (ic + 1) * P, :], ot)

    nc.compile()
    return nc


def _get_nc():
    if "nc" not in _CACHED:
        _CACHED["nc"] = _build_program()
    return _CACHED["nc"]


def _make_in_maps(query, target, wq, bq, wk, bk, wv, bv, wo, bo):
    query = np.asarray(query, dtype=np.float32)
    target = np.asarray(target, dtype=np.float32)
    consts = {
        "wq": np.asarray(wq, np.float32), "bq": np.asarray(bq, np.float32),
        "wk": np.asarray(wk, np.float32), "bk": np.asarray(bk, np.float32),
        "wv": np.asarray(wv, np.float32), "bv": np.asarray(bv, np.float32),
        "wo": np.asarray(wo, np.float32), "bo": np.asarray(bo, np.float32),
    }
    in_maps = []
    for core in range(8):
        b, h = divmod(core, 2)
        in_maps.append({
            "query": np.ascontiguousarray(query[b, h * LQH:(h + 1) * LQH]),
            # faithful to the torch reshape: raw reinterpret of [512, 4096]
            "target": np.ascontiguousarray(target[b]).reshape(S, D),
            **consts,
        })
    return in_maps


def kernel(query, target, wq, bq, wk, bk, wv, bv, wo, bo):
    from concourse import bass_utils
    nc = _get_nc()
    in_maps = _make_in_maps(query, target, wq, bq, wk, bk, wv, bv, wo, bo)
    res = bass_utils.run_bass_kernel_spmd(nc, in_maps, core_ids=list(range(8)))
    out = np.empty((B, LQ, D), np.float32)
    for core in range(8):
        b, h = divmod(core, 2)
        out[b, h * LQH:(h + 1) * LQH] = res.results[core]["out"]
    return out


# revision 4
# speedup vs baseline: 1.0935x; 1.0935x over previous
"""Trainium2 Bass kernel for nn_CrossAttention (B=4, LQ=4096, S=4096, D=512).

Sharding: data-parallel over (batch, query-half): core = 2*b + half.
Each core computes cross-attention for one batch element and 2048 query rows.
K/V projections are recomputed by both cores of a pair (cheap relative to
the attention matmuls).

All heavy matmuls run in bf16 with fp32 PSUM accumulation. Softmax skips the
max-subtraction (scores are bounded ~ +-30 here, exp stays in fp32 range) and
the normalization division is deferred to after the context matmul.

Big activations are split into per-block subtiles so the Tile scheduler can
overlap the load/transpose, projection, and attention stages.
"""

import numpy as np

B, LQ, S = 4, 4096, 4096
D = 512          # SRC == TGT == 512
P = 128
LQH = LQ // 2    # 2048 query rows per core
DC = D // P      # 4 chunks of the feature dims
SC = S // P      # 32 s-chunks
IB = 512         # block width (query i / kv s) per subtile
NB = LQH // IB   # 4 query blocks
KB = S // IB     # 8 kv blocks

_CACHED = {}


def _build_program():
    import concourse.bass as bass
    import concourse.mybir as mybir
    import concourse.tile as tile
    from concourse import bacc
    from concourse.masks import make_identity
    from contextlib import ExitStack

    f32 = mybir.dt.float32
    bf16 = mybir.dt.bfloat16
    AF = mybir.ActivationFunctionType
    OP = mybir.AluOpType

    nc = bacc.Bacc("TRN2", target_bir_lowering=False, debug=False, num_devices=8)

    query = nc.dram_tensor("query", [LQH, D], f32, kind="ExternalInput").ap()
    target = nc.dram_tensor("target", [S, D], f32, kind="ExternalInput").ap()
    w_dram = {}
    b_dram = {}
    for nm in ("wq", "wk", "wv", "wo"):
        w_dram[nm] = nc.dram_tensor(nm, [D, D], f32, kind="ExternalInput").ap()
    for nm in ("bq", "bk", "bv", "bo"):
        b_dram[nm] = nc.dram_tensor(nm, [D], f32, kind="ExternalInput").ap()
    out_dram = nc.dram_tensor("out", [LQH, D], f32, kind="ExternalOutput").ap()

    with tile.TileContext(nc) as tc, ExitStack() as ctx:
        const = ctx.enter_context(tc.tile_pool(name="const", bufs=1))
        big = ctx.enter_context(tc.tile_pool(name="big", bufs=1))
        wstage = ctx.enter_context(tc.tile_pool(name="wstage", bufs=1))
        ld = ctx.enter_context(tc.tile_pool(name="ld", bufs=4))
        ptp = ctx.enter_context(tc.tile_pool(name="ptp", bufs=4))
        ctxp = ctx.enter_context(tc.tile_pool(name="ctxp", bufs=2))
        outp = ctx.enter_context(tc.tile_pool(name="outp", bufs=2))
        smallp = ctx.enter_context(tc.tile_pool(name="smallp", bufs=2))
        ps_small = ctx.enter_context(tc.tile_pool(name="ps_small", bufs=3, space="PSUM"))
        ps_acc = ctx.enter_context(tc.tile_pool(name="ps_acc", bufs=4, space="PSUM"))
        ps_rs = ctx.enter_context(tc.tile_pool(name="ps_rs", bufs=1, space="PSUM"))

        # ---- constants ----
        ident = const.tile([P, P], f32, tag="ident", name="ident")
        make_identity(nc, ident)

        ones_col = const.tile([P, 1], bf16, tag="ones_col", name="ones_col")
        nc.vector.memset(ones_col, 1.0)
        ones_row = const.tile([1, P], f32, tag="ones_row", name="ones_row")
        nc.vector.memset(ones_row, 1.0)

        # ---- stage A (query): load + transpose to bf16 [d-part, i] ----
        qinT = [big.tile([P, DC, IB], bf16, tag=f"qinT{i}", name=f"qinT{i}")
                for i in range(NB)]

        def load_transposed(src, row0, dstT, col0, nchunks):
            # transpose nchunks 128-row chunks of src into dstT[:, :, col...]
            for cc in range(nchunks):
                r = row0 + cc * P
                chunk = ld.tile([P, D], f32, tag="in_chunk", name=f"chk_{dstT.name}_{cc}")
                nc.sync.dma_start(chunk, src[r:r + P, :])
                ps = ps_small.tile([P, D], f32, tag="ps_small", name=f"pst_{dstT.name}_{cc}")
                for dc in range(DC):
                    nc.tensor.transpose(ps[:, dc * P:(dc + 1) * P],
                                        chunk[:, dc * P:(dc + 1) * P], ident)
                c = col0 + cc * P
                nc.scalar.activation(dstT[:, :, c:c + P],
                                     ps.rearrange("p (c q) -> p c q", c=DC),
                                     AF.Copy)

        for ibk in range(NB):
            load_transposed(query, ibk * IB, qinT[ibk], 0, IB // P)

        # ---- weights -> bf16 [P, DC, D] (contraction dim on partitions) ----
        w_bf = {}
        for nm in ("wq", "wk", "wv", "wo"):
            wf = wstage.tile([P, DC, D], f32, tag="w_stage", name=f"{nm}_f32")
            nc.sync.dma_start(wf, w_dram[nm].rearrange("(c p) n -> p c n", p=P))
            wb = const.tile([P, DC, D], bf16, tag=f"w_{nm}", name=f"{nm}_bf")
            nc.vector.tensor_copy(out=wb, in_=wf)
            w_bf[nm] = wb

        b_col = {}
        for nm in ("bq", "bk"):
            bc = const.tile([P, DC], f32, tag=f"col_{nm}", name=f"{nm}_col")
            nc.gpsimd.dma_start(out=bc, in_=b_dram[nm].rearrange("(c p) -> p c", p=P))
            b_col[nm] = bc
        b_rep = {}
        for nm in ("bv", "bo"):
            br = const.tile([P, D], f32, tag=f"rep_{nm}", name=f"{nm}_rep")
            src = b_dram[nm]
            bcast = bass.AP(tensor=src.tensor, offset=src.offset,
                            ap=[[0, P]] + list(src.ap))
            nc.gpsimd.dma_start(out=br, in_=bcast)
            b_rep[nm] = br

        # ---- qT projection per block: qT[t, i] = wq.T @ qin + bq ----
        qT = [big.tile([P, DC, IB], bf16, tag=f"qT{i}", name=f"qT{i}")
              for i in range(NB)]
        for ibk in range(NB):
            for tcc in range(DC):
                ps = ps_small.tile([P, IB], f32, tag="ps_small", name=f"psq_{tcc}_{ibk}")
                for dc in range(DC):
                    nc.tensor.matmul(ps, w_bf["wq"][:, dc, tcc * P:(tcc + 1) * P],
                                     qinT[ibk][:, dc, :],
                                     start=(dc == 0), stop=(dc == DC - 1))
                nc.vector.tensor_tensor(qT[ibk][:, tcc, :], ps,
                                        b_col["bq"][:, tcc:tcc + 1].to_broadcast([P, IB]),
                                        OP.add)

        # ---- stage A (target) + kT / v projections, per 512-s block ----
        tgtT = [big.tile([P, DC, IB], bf16, tag=f"tgtT{i}", name=f"tgtT{i}")
                for i in range(KB)]
        kT = [big.tile([P, DC, IB], bf16, tag=f"kT{i}", name=f"kT{i}")
              for i in range(KB)]
        vv = [big.tile([P, IB // P, D], bf16, tag=f"vv{i}", name=f"vv{i}")
              for i in range(KB)]

        for sb in range(KB):
            load_transposed(target, sb * IB, tgtT[sb], 0, IB // P)
            # kT[t, s] = wk.T @ tgtT + bk
            for tcc in range(DC):
                ps = ps_small.tile([P, IB], f32, tag="ps_small", name=f"psk_{tcc}_{sb}")
                for dc in range(DC):
                    nc.tensor.matmul(ps, w_bf["wk"][:, dc, tcc * P:(tcc + 1) * P],
                                     tgtT[sb][:, dc, :],
                                     start=(dc == 0), stop=(dc == DC - 1))
                nc.vector.tensor_tensor(kT[sb][:, tcc, :], ps,
                                        b_col["bk"][:, tcc:tcc + 1].to_broadcast([P, IB]),
                                        OP.add)
            # v[s, d'] = tgt @ wv + bv
            for sl in range(IB // P):
                ps = ps_small.tile([P, D], f32, tag="ps_small", name=f"psv_{sb}_{sl}")
                for dc in range(DC):
                    nc.tensor.matmul(ps, tgtT[sb][:, dc, sl * P:(sl + 1) * P],
                                     w_bf["wv"][:, dc, :],
                                     start=(dc == 0), stop=(dc == DC - 1))
                nc.vector.tensor_tensor(vv[sb][:, sl, :], ps, b_rep["bv"], OP.add)

        # ---- stage C: attention + output projection, per 512-wide i block ----
        for ib in range(NB):
            ctx_ps = [ps_acc.tile([P, IB], f32, tag="ps_acc", name=f"ctx_{ib}_{d}")
                      for d in range(DC)]
            rs_ps = ps_rs.tile([1, IB], f32, tag="rs", name=f"rs_{ib}")

            for scc in range(SC):
                g, sl = divmod(scc, IB // P)
                pt_ps = ps_small.tile([P, IB], f32, tag="ps_small", name=f"pt_{ib}_{scc}")
                for tcc in range(DC):
                    nc.tensor.matmul(pt_ps, kT[g][:, tcc, sl * P:(sl + 1) * P],
                                     qT[ib][:, tcc, :],
                                     start=(tcc == 0), stop=(tcc == DC - 1))
                pt_exp = ptp.tile([P, IB], bf16, tag="pt_exp", name=f"pte_{ib}_{scc}")
                nc.scalar.activation(pt_exp, pt_ps, AF.Exp)
                nc.tensor.matmul(rs_ps, ones_col, pt_exp,
                                 start=(scc == 0), stop=(scc == SC - 1))
                for dpc in range(DC):
                    nc.tensor.matmul(ctx_ps[dpc], vv[g][:, sl, dpc * P:(dpc + 1) * P],
                                     pt_exp, start=(scc == 0), stop=(scc == SC - 1))

            recip = smallp.tile([1, IB], f32, tag="recip", name=f"recip_{ib}")
            nc.vector.reciprocal(recip, rs_ps)
            rb_ps = ps_small.tile([P, IB], f32, tag="ps_small", name=f"rb_{ib}")
            nc.tensor.matmul(rb_ps, ones_row, recip, start=True, stop=True)
            rb_sb = smallp.tile([P, IB], f32, tag="rb_sb", name=f"rb_sb_{ib}")
            nc.scalar.activation(rb_sb, rb_ps, AF.Copy)

            ctxT = ctxp.tile([P, DC, IB], bf16, tag="ctxT", name=f"ctxT_{ib}")
            for dpc in range(DC):
                nc.vector.tensor_tensor(ctxT[:, dpc, :], ctx_ps[dpc], rb_sb, OP.mult)

            for ic in range(DC):
                op_ps = ps_acc.tile([P, D], f32, tag="ps_acc", name=f"op_{ib}_{ic}")
                for dpc in range(DC):
                    nc.tensor.matmul(op_ps, ctxT[:, dpc, ic * P:(ic + 1) * P],
                                     w_bf["wo"][:, dpc, :],
                                     start=(dpc == 0), stop=(dpc == DC - 1))
                ot = outp.tile([P, D], f32, tag="out_t", name=f"ot_{ib}_{ic}")
                nc.vector.tensor_tensor(ot, op_ps, b_rep["bo"], OP.add)
                nc.sync.dma_start(out_dram[ib * IB + ic * P: ib * IB + (ic + 1) * P, :], ot)

    nc.compile()
    return nc


def _get_nc():
    if "nc" not in _CACHED:
        _CACHED["nc"] = _build_program()
    return _CACHED["nc"]


def _make_in_maps(query, target, wq, bq, wk, bk, wv, bv, wo, bo):
    query = np.asarray(query, dtype=np.float32)
    target = np.asarray(target, dtype=np.float32)
    consts = {
        "wq": np.asarray(wq, np.float32), "bq": np.asarray(bq, np.float32),
        "wk": np.asarray(wk, np.float32), "bk": np.asarray(bk, np.float32),
        "wv": np.asarray(wv, np.float32), "bv": np.asarray(bv, np.float32),
        "wo": np.asarray(wo, np.float32), "bo": np.asarray(bo, np.float32),
    }
    in_maps = []
    for core in range(8):
        b, h = divmod(core, 2)
        in_maps.append({
            "query": np.ascontiguousarray(query[b, h * LQH:(h + 1) * LQH]),
            # faithful to the torch reshape: raw reinterpret of [512, 4096]
            "target": np.ascontiguousarray(target[b]).reshape(S, D),
            **consts,
        })
    return in_maps


def kernel(query, target, wq, bq, wk, bk, wv, bv, wo, bo):
    from concourse import bass_utils
    nc = _get_nc()
    in_maps = _make_in_maps(query, target, wq, bq, wk, bk, wv, bv, wo, bo)
    res = bass_utils.run_bass_kernel_spmd(nc, in_maps, core_ids=list(range(8)))
    out = np.empty((B, LQ, D), np.float32)
    for core in range(8):
        b, h = divmod(core, 2)
        out[b, h * LQH:(h + 1) * LQH] = res.results[core]["out"]
    return out


# revision 6
# speedup vs baseline: 1.1466x; 1.0485x over previous
"""Trainium2 Bass kernel for nn_CrossAttention (B=4, LQ=4096, S=4096, D=512).

Sharding: data-parallel over (batch, query-half): core = 2*b + half.
Each core computes cross-attention for one batch element and 2048 query rows.
K/V projections are recomputed by both cores of a pair (cheap relative to
the attention matmuls).

All heavy matmuls run in bf16 with fp32 PSUM accumulation. Softmax skips the
max-subtraction (scores are bounded ~ +-30 here, exp stays in fp32 range).
The softmax normalization is applied at the very end: the context and output
projection run on unnormalized sums, and the output tiles are scaled by the
per-row reciprocal (obtained as a per-partition column via a PE transpose of
the row-sum reciprocal), which keeps the block tail off the PE critical path.
"""

import numpy as np

B, LQ, S = 4, 4096, 4096
D = 512          # SRC == TGT == 512
P = 128
LQH = LQ // 2    # 2048 query rows per core
DC = D // P      # 4 chunks of the feature dims
SC = S // P      # 32 s-chunks
IB = 512         # block width (query i / kv s) per subtile
NB = LQH // IB   # 4 query blocks
KB = S // IB     # 8 kv blocks

_CACHED = {}


def _build_program():
    import concourse.bass as bass
    import concourse.mybir as mybir
    import concourse.tile as tile
    from concourse import bacc
    from concourse.masks import make_identity
    from contextlib import ExitStack

    f32 = mybir.dt.float32
    bf16 = mybir.dt.bfloat16
    AF = mybir.ActivationFunctionType
    OP = mybir.AluOpType

    nc = bacc.Bacc("TRN2", target_bir_lowering=False, debug=False, num_devices=8)

    query = nc.dram_tensor("query", [LQH, D], f32, kind="ExternalInput").ap()
    target = nc.dram_tensor("target", [S, D], f32, kind="ExternalInput").ap()
    w_dram = {}
    b_dram = {}
    for nm in ("wq", "wk", "wv", "wo"):
        w_dram[nm] = nc.dram_tensor(nm, [D, D], f32, kind="ExternalInput").ap()
    for nm in ("bq", "bk", "bv", "bo"):
        b_dram[nm] = nc.dram_tensor(nm, [D], f32, kind="ExternalInput").ap()
    out_dram = nc.dram_tensor("out", [LQH, D], f32, kind="ExternalOutput").ap()

    with tile.TileContext(nc) as tc, ExitStack() as ctx:
        const = ctx.enter_context(tc.tile_pool(name="const", bufs=1))
        big = ctx.enter_context(tc.tile_pool(name="big", bufs=1))
        wstage = ctx.enter_context(tc.tile_pool(name="wstage", bufs=1))
        ld = ctx.enter_context(tc.tile_pool(name="ld", bufs=6))
        ptp = ctx.enter_context(tc.tile_pool(name="ptp", bufs=4))
        ctxp = ctx.enter_context(tc.tile_pool(name="ctxp", bufs=2))
        outp = ctx.enter_context(tc.tile_pool(name="outp", bufs=2))
        smallp = ctx.enter_context(tc.tile_pool(name="smallp", bufs=2))
        ps_small = ctx.enter_context(tc.tile_pool(name="ps_small", bufs=3, space="PSUM"))
        ps_acc = ctx.enter_context(tc.tile_pool(name="ps_acc", bufs=4, space="PSUM"))
        ps_rs = ctx.enter_context(tc.tile_pool(name="ps_rs", bufs=1, space="PSUM"))

        # ---- constants (tiny, first so nothing waits on them) ----
        ident_b = const.tile([P, P], bf16, tag="ident_b", name="ident_b")
        make_identity(nc, ident_b)
        ident_f = const.tile([P, P], f32, tag="ident_f", name="ident_f")
        make_identity(nc, ident_f)

        ones_col = const.tile([P, 1], bf16, tag="ones_col", name="ones_col")
        nc.vector.memset(ones_col, 1.0)

        b_col = {}
        for nm in ("bq", "bk"):
            bc = const.tile([P, DC], f32, tag=f"col_{nm}", name=f"{nm}_col")
            nc.gpsimd.dma_start(out=bc, in_=b_dram[nm].rearrange("(c p) -> p c", p=P))
            b_col[nm] = bc
        b_rep = {}
        for nm in ("bv", "bo"):
            br = const.tile([P, D], f32, tag=f"rep_{nm}", name=f"{nm}_rep")
            src = b_dram[nm]
            bcast = bass.AP(tensor=src.tensor, offset=src.offset,
                            ap=[[0, P]] + list(src.ap))
            nc.gpsimd.dma_start(out=br, in_=bcast)
            b_rep[nm] = br

        # staging row for the reciprocal transpose: row 0 live, rows 1.. zero
        rstage = const.tile([P, IB], f32, tag="rstage", name="rstage")
        nc.vector.memset(rstage, 0.0)

        w_bf = {}

        def load_weight(nm):
            wf = wstage.tile([P, DC, D], f32, tag="w_stage", name=f"{nm}_f32")
            nc.sync.dma_start(wf, w_dram[nm].rearrange("(c p) n -> p c n", p=P))
            wb = const.tile([P, DC, D], bf16, tag=f"w_{nm}", name=f"{nm}_bf")
            nc.vector.tensor_copy(out=wb, in_=wf)
            w_bf[nm] = wb

        def load_transposed(src, row0, dstT):
            # cast 128-row chunks to bf16 and PE-transpose into dstT[:, dc, :]
            for cc in range(IB // P):
                r = row0 + cc * P
                chunk = ld.tile([P, D], f32, tag="in_chunk", name=f"chk_{dstT.name}_{cc}")
                nc.sync.dma_start(chunk, src[r:r + P, :])
                cast = ld.tile([P, D], bf16, tag="in_cast", name=f"cst_{dstT.name}_{cc}")
                nc.vector.tensor_copy(out=cast, in_=chunk)
                psv = ps_acc.tile([P, D], bf16, tag="ps_acc", name=f"pst_{dstT.name}_{cc}")
                for dc in range(DC):
                    nc.tensor.transpose(psv[:, dc * P:(dc + 1) * P],
                                        cast[:, dc * P:(dc + 1) * P], ident_b)
                c = cc * P
                nc.scalar.activation(dstT[:, :, c:c + P],
                                     psv.rearrange("p (c q) -> p c q", c=DC),
                                     AF.Copy)

        # ---- stage A+B (query side) ----
        load_weight("wq")
        qinT = [big.tile([P, DC, IB], bf16, tag=f"qinT{i}", name=f"qinT{i}")
                for i in range(NB)]
        qT = [big.tile([P, DC, IB], bf16, tag=f"qT{i}", name=f"qT{i}")
              for i in range(NB)]
        for ibk in range(NB):
            load_transposed(query, ibk * IB, qinT[ibk])
            for tcc in range(DC):
                ps = ps_small.tile([P, IB], f32, tag="ps_small", name=f"psq_{tcc}_{ibk}")
                for dc in range(DC):
                    nc.tensor.matmul(ps, w_bf["wq"][:, dc, tcc * P:(tcc + 1) * P],
                                     qinT[ibk][:, dc, :],
                                     start=(dc == 0), stop=(dc == DC - 1))
                nc.vector.tensor_tensor(qT[ibk][:, tcc, :], ps,
                                        b_col["bq"][:, tcc:tcc + 1].to_broadcast([P, IB]),
                                        OP.add)

        # ---- stage A+B (target side) ----
        load_weight("wk")
        load_weight("wv")
        tgtT = [big.tile([P, DC, IB], bf16, tag=f"tgtT{i}", name=f"tgtT{i}")
                for i in range(KB)]
        kT = [big.tile([P, DC, IB], bf16, tag=f"kT{i}", name=f"kT{i}")
              for i in range(KB)]
        vv = [big.tile([P, IB // P, D], bf16, tag=f"vv{i}", name=f"vv{i}")
              for i in range(KB)]

        for sb in range(KB):
            load_transposed(target, sb * IB, tgtT[sb])
            for tcc in range(DC):
                ps = ps_small.tile([P, IB], f32, tag="ps_small", name=f"psk_{tcc}_{sb}")
                for dc in range(DC):
                    nc.tensor.matmul(ps, w_bf["wk"][:, dc, tcc * P:(tcc + 1) * P],
                                     tgtT[sb][:, dc, :],
                                     start=(dc == 0), stop=(dc == DC - 1))
                nc.vector.tensor_tensor(kT[sb][:, tcc, :], ps,
                                        b_col["bk"][:, tcc:tcc + 1].to_broadcast([P, IB]),
                                        OP.add)
            for sl in range(IB // P):
                ps = ps_small.tile([P, D], f32, tag="ps_small", name=f"psv_{sb}_{sl}")
                for dc in range(DC):
                    nc.tensor.matmul(ps, tgtT[sb][:, dc, sl * P:(sl + 1) * P],
                                     w_bf["wv"][:, dc, :],
                                     start=(dc == 0), stop=(dc == DC - 1))
                nc.vector.tensor_tensor(vv[sb][:, sl, :], ps, b_rep["bv"], OP.add)

        load_weight("wo")

        # ---- stage C: attention + output projection, per 512-wide i block ----
        for ib in range(NB):
            ctx_ps = [ps_acc.tile([P, IB], f32, tag="ps_acc", name=f"ctx_{ib}_{d}")
                      for d in range(DC)]
            rs_ps = ps_rs.tile([1, IB], f32, tag="rs", name=f"rs_{ib}")

            for scc in range(SC):
                g, sl = divmod(scc, IB // P)
                pt_ps = ps_small.tile([P, IB], f32, tag="ps_small", name=f"pt_{ib}_{scc}")
                for tcc in range(DC):
                    nc.tensor.matmul(pt_ps, kT[g][:, tcc, sl * P:(sl + 1) * P],
                                     qT[ib][:, tcc, :],
                                     start=(tcc == 0), stop=(tcc == DC - 1))
                pt_exp = ptp.tile([P, IB], bf16, tag="pt_exp", name=f"pte_{ib}_{scc}")
                nc.scalar.activation(pt_exp, pt_ps, AF.Exp)
                nc.tensor.matmul(rs_ps, ones_col, pt_exp,
                                 start=(scc == 0), stop=(scc == SC - 1))
                for dpc in range(DC):
                    nc.tensor.matmul(ctx_ps[dpc], vv[g][:, sl, dpc * P:(dpc + 1) * P],
                                     pt_exp, start=(scc == 0), stop=(scc == SC - 1))

            # unnormalized context -> bf16 (no dependency on the reciprocal)
            ctxT = ctxp.tile([P, DC, IB], bf16, tag="ctxT", name=f"ctxT_{ib}")
            for dpc in range(DC):
                nc.vector.tensor_copy(out=ctxT[:, dpc, :], in_=ctx_ps[dpc])

            # per-row reciprocal as per-partition columns (PE transpose trick)
            recip = smallp.tile([1, IB], f32, tag="recip", name=f"recip_{ib}")
            nc.vector.reciprocal(recip, rs_ps)
            nc.vector.tensor_copy(out=rstage[0:1, :], in_=recip)
            rt_ps = ps_small.tile([P, IB], f32, tag="ps_small", name=f"rt_{ib}")
            for ic in range(DC):
                nc.tensor.transpose(rt_ps[:, ic * P:(ic + 1) * P],
                                    rstage[:, ic * P:(ic + 1) * P], ident_f)
            rc_sb = smallp.tile([P, DC], f32, tag="rc_sb", name=f"rc_{ib}")
            nc.scalar.activation(rc_sb,
                                 rt_ps.rearrange("p (c q) -> p c q", c=DC)[:, :, 0],
                                 AF.Copy)

            for ic in range(DC):
                op_ps = ps_acc.tile([P, D], f32, tag="ps_acc", name=f"op_{ib}_{ic}")
                for dpc in range(DC):
                    nc.tensor.matmul(op_ps, ctxT[:, dpc, ic * P:(ic + 1) * P],
                                     w_bf["wo"][:, dpc, :],
                                     start=(dpc == 0), stop=(dpc == DC - 1))
                # scale rows by 1/rowsum on ACT, then add bias on DVE
                ot_s = outp.tile([P, D], f32, tag="out_s", name=f"ots_{ib}_{ic}")
                nc.scalar.activation(ot_s, op_ps, AF.Copy,
                                     scale=rc_sb[:, ic:ic + 1])
                ot = outp.tile([P, D], f32, tag="out_t", name=f"ot_{ib}_{ic}")
                nc.vector.tensor_tensor(ot, ot_s, b_rep["bo"], OP.add)
                nc.sync.dma_start(out_dram[ib * IB + ic * P: ib * IB + (ic + 1) * P, :], ot)

    nc.compile()
    return nc


def _get_nc():
    if "nc" not in _CACHED:
        _CACHED["nc"] = _build_program()
    return _CACHED["nc"]


def _make_in_maps(query, target, wq, bq, wk, bk, wv, bv, wo, bo):
    query = np.asarray(query, dtype=np.float32)
    target = np.asarray(target, dtype=np.float32)
    consts = {
        "wq": np.asarray(wq, np.float32), "bq": np.asarray(bq, np.float32),
        "wk": np.asarray(wk, np.float32), "bk": np.asarray(bk, np.float32),
        "wv": np.asarray(wv, np.float32), "bv": np.asarray(bv, np.float32),
        "wo": np.asarray(wo, np.float32), "bo": np.asarray(bo, np.float32),
    }
    in_maps = []
    for core in range(8):
        b, h = divmod(core, 2)
        in_maps.append({
            "query": np.ascontiguousarray(query[b, h * LQH:(h + 1) * LQH]),
            # faithful to the torch reshape: raw reinterpret of [512, 4096]
            "target": np.ascontiguousarray(target[b]).reshape(S, D),
            **consts,
        })
    return in_maps


def kernel(query, target, wq, bq, wk, bk, wv, bv, wo, bo):
    from concourse import bass_utils
    nc = _get_nc()
    in_maps = _make_in_maps(query, target, wq, bq, wk, bk, wv, bv, wo, bo)
    res = bass_utils.run_bass_kernel_spmd(nc, in_maps, core_ids=list(range(8)))
    out = np.empty((B, LQ, D), np.float32)
    for core in range(8):
        b, h = divmod(core, 2)
        out[b, h * LQH:(h + 1) * LQH] = res.results[core]["out"]
    return out


# revision 9
# speedup vs baseline: 1.1975x; 1.0444x over previous
"""Trainium2 Bass kernel for nn_CrossAttention (B=4, LQ=4096, S=4096, D=512).

Sharding: data-parallel over (batch, query-half): core = 2*b + half.
Each core computes cross-attention for one batch element and 2048 query rows.
K/V projections are recomputed by both cores of a pair (cheap relative to
the attention matmuls).

All heavy matmuls run in bf16 with fp32 PSUM accumulation. Softmax skips the
max-subtraction (scores are bounded ~ +-30 here, exp stays in fp32 range).
The softmax normalization is applied at the very end: the context and output
projection run on unnormalized sums, and the output tiles are scaled by the
per-row reciprocal (obtained as a per-partition column via a PE transpose of
the row-sum reciprocal), which keeps the block tail off the PE critical path.
"""

import numpy as np

B, LQ, S = 4, 4096, 4096
D = 512          # SRC == TGT == 512
P = 128
LQH = LQ // 2    # 2048 query rows per core
DC = D // P      # 4 chunks of the feature dims
SC = S // P      # 32 s-chunks
IB = 512         # block width (query i / kv s) per subtile
NB = LQH // IB   # 4 query blocks
KB = S // IB     # 8 kv blocks

_CACHED = {}


def _build_program():
    import concourse.bass as bass
    import concourse.mybir as mybir
    import concourse.tile as tile
    from concourse import bacc
    from concourse.masks import make_identity
    from contextlib import ExitStack

    f32 = mybir.dt.float32
    bf16 = mybir.dt.bfloat16
    AF = mybir.ActivationFunctionType
    OP = mybir.AluOpType

    nc = bacc.Bacc("TRN2", target_bir_lowering=False, debug=False, num_devices=8)

    query = nc.dram_tensor("query", [LQH, D], f32, kind="ExternalInput").ap()
    target = nc.dram_tensor("target", [S, D], f32, kind="ExternalInput").ap()
    w_dram = {}
    b_dram = {}
    for nm in ("wq", "wk", "wv", "wo"):
        w_dram[nm] = nc.dram_tensor(nm, [D, D], f32, kind="ExternalInput").ap()
    for nm in ("bq", "bk", "bv", "bo"):
        b_dram[nm] = nc.dram_tensor(nm, [D], f32, kind="ExternalInput").ap()
    out_dram = nc.dram_tensor("out", [LQH, D], f32, kind="ExternalOutput").ap()

    with tile.TileContext(nc) as tc, ExitStack() as ctx:
        const = ctx.enter_context(tc.tile_pool(name="const", bufs=1))
        big = ctx.enter_context(tc.tile_pool(name="big", bufs=1))
        wstage = ctx.enter_context(tc.tile_pool(name="wstage", bufs=1))
        ld = ctx.enter_context(tc.tile_pool(name="ld", bufs=6))
        ptp = ctx.enter_context(tc.tile_pool(name="ptp", bufs=6))
        ctxp = ctx.enter_context(tc.tile_pool(name="ctxp", bufs=2))
        outp = ctx.enter_context(tc.tile_pool(name="outp", bufs=2))
        smallp = ctx.enter_context(tc.tile_pool(name="smallp", bufs=2))
        ps_small = ctx.enter_context(tc.tile_pool(name="ps_small", bufs=3, space="PSUM"))
        ps_acc = ctx.enter_context(tc.tile_pool(name="ps_acc", bufs=4, space="PSUM"))
        ps_rs = ctx.enter_context(tc.tile_pool(name="ps_rs", bufs=1, space="PSUM"))

        # ---- constants (tiny, first so nothing waits on them) ----
        ident_b = const.tile([P, P], bf16, tag="ident_b", name="ident_b")
        make_identity(nc, ident_b)
        ident_f = const.tile([P, P], f32, tag="ident_f", name="ident_f")
        make_identity(nc, ident_f)

        ones_col = const.tile([P, 1], bf16, tag="ones_col", name="ones_col")
        nc.vector.memset(ones_col, 1.0)

        b_col = {}
        for nm in ("bq", "bk"):
            bc = const.tile([P, DC], f32, tag=f"col_{nm}", name=f"{nm}_col")
            nc.gpsimd.dma_start(out=bc, in_=b_dram[nm].rearrange("(c p) -> p c", p=P))
            b_col[nm] = bc
        b_rep = {}
        for nm in ("bv", "bo"):
            br = const.tile([P, D], f32, tag=f"rep_{nm}", name=f"{nm}_rep")
            src = b_dram[nm]
            bcast = bass.AP(tensor=src.tensor, offset=src.offset,
                            ap=[[0, P]] + list(src.ap))
            nc.gpsimd.dma_start(out=br, in_=bcast)
            b_rep[nm] = br

        # staging row for the reciprocal transpose: row 0 live, rows 1.. zero
        rstage = const.tile([P, IB], f32, tag="rstage", name="rstage")
        nc.vector.memset(rstage, 0.0)

        w_bf = {}

        def load_weight(nm):
            wf = wstage.tile([P, DC, D], f32, tag="w_stage", name=f"{nm}_f32")
            nc.sync.dma_start(wf, w_dram[nm].rearrange("(c p) n -> p c n", p=P))
            wb = const.tile([P, DC, D], bf16, tag=f"w_{nm}", name=f"{nm}_bf")
            nc.vector.tensor_copy(out=wb, in_=wf)
            w_bf[nm] = wb

        def load_transposed(src, row0, dstT):
            # cast 128-row chunks to bf16 and PE-transpose into dstT[:, dc, :]
            for cc in range(IB // P):
                r = row0 + cc * P
                chunk = ld.tile([P, D], f32, tag="in_chunk", name=f"chk_{dstT.name}_{cc}")
                nc.sync.dma_start(chunk, src[r:r + P, :])
                cast = ld.tile([P, D], bf16, tag="in_cast", name=f"cst_{dstT.name}_{cc}")
                nc.vector.tensor_copy(out=cast, in_=chunk)
                psv = ps_acc.tile([P, D], bf16, tag="ps_acc", name=f"pst_{dstT.name}_{cc}")
                for dc in range(DC):
                    nc.tensor.transpose(psv[:, dc * P:(dc + 1) * P],
                                        cast[:, dc * P:(dc + 1) * P], ident_b)
                c = cc * P
                nc.scalar.activation(dstT[:, :, c:c + P],
                                     psv.rearrange("p (c q) -> p c q", c=DC),
                                     AF.Copy)

        # ---- stage A+B (query side) ----
        qinT = [big.tile([P, DC, IB], bf16, tag=f"qinT{i}", name=f"qinT{i}")
                for i in range(NB)]
        qT = [big.tile([P, DC, IB], bf16, tag=f"qT{i}", name=f"qT{i}")
              for i in range(NB)]
        for ibk in range(NB):
            load_transposed(query, ibk * IB, qinT[ibk])
            if ibk == 0:
                # after the first chunk DMAs so transposes start immediately
                load_weight("wq")
            for tcc in range(DC):
                ps = ps_small.tile([P, IB], f32, tag="ps_small", name=f"psq_{tcc}_{ibk}")
                for dc in range(DC):
                    nc.tensor.matmul(ps, w_bf["wq"][:, dc, tcc * P:(tcc + 1) * P],
                                     qinT[ibk][:, dc, :],
                                     start=(dc == 0), stop=(dc == DC - 1))
                nc.vector.tensor_tensor(qT[ibk][:, tcc, :], ps,
                                        b_col["bq"][:, tcc:tcc + 1].to_broadcast([P, IB]),
                                        OP.add)

        # ---- stage A+B (target side) ----
        load_weight("wk")
        load_weight("wv")
        tgtT = [big.tile([P, DC, IB], bf16, tag=f"tgtT{i}", name=f"tgtT{i}")
                for i in range(KB)]
        kT = [big.tile([P, DC, IB], bf16, tag=f"kT{i}", name=f"kT{i}")
              for i in range(KB)]
        vv = [big.tile([P, IB // P, D], bf16, tag=f"vv{i}", name=f"vv{i}")
              for i in range(KB)]

        for sb in range(KB):
            load_transposed(target, sb * IB, tgtT[sb])
            for tcc in range(DC):
                ps = ps_small.tile([P, IB], f32, tag="ps_small", name=f"psk_{tcc}_{sb}")
                for dc in range(DC):
                    nc.tensor.matmul(ps, w_bf["wk"][:, dc, tcc * P:(tcc + 1) * P],
                                     tgtT[sb][:, dc, :],
                                     start=(dc == 0), stop=(dc == DC - 1))
                nc.vector.tensor_tensor(kT[sb][:, tcc, :], ps,
                                        b_col["bk"][:, tcc:tcc + 1].to_broadcast([P, IB]),
                                        OP.add)
            for sl in range(IB // P):
                ps = ps_small.tile([P, D], f32, tag="ps_small", name=f"psv_{sb}_{sl}")
                for dc in range(DC):
                    nc.tensor.matmul(ps, tgtT[sb][:, dc, sl * P:(sl + 1) * P],
                                     w_bf["wv"][:, dc, :],
                                     start=(dc == 0), stop=(dc == DC - 1))
                nc.vector.tensor_tensor(vv[sb][:, sl, :], ps, b_rep["bv"], OP.add)

        load_weight("wo")

        # ---- stage C: attention + output projection, per 512-wide i block ----
        for ib in range(NB):
            ctx_ps = [ps_acc.tile([P, IB], f32, tag="ps_acc", name=f"ctx_{ib}_{d}")
                      for d in range(DC)]
            rs_ps = ps_rs.tile([1, IB], f32, tag="rs", name=f"rs_{ib}")

            for scc in range(SC):
                g, sl = divmod(scc, IB // P)
                pt_ps = ps_small.tile([P, IB], f32, tag="ps_small", name=f"pt_{ib}_{scc}")
                for tcc in range(DC):
                    nc.tensor.matmul(pt_ps, kT[g][:, tcc, sl * P:(sl + 1) * P],
                                     qT[ib][:, tcc, :],
                                     start=(tcc == 0), stop=(tcc == DC - 1))
                pt_exp = ptp.tile([P, IB], bf16, tag="pt_exp", name=f"pte_{ib}_{scc}")
                nc.scalar.activation(pt_exp, pt_ps, AF.Exp)
                nc.tensor.matmul(rs_ps, ones_col, pt_exp,
                                 start=(scc == 0), stop=(scc == SC - 1))
                for dpc in range(DC):
                    nc.tensor.matmul(ctx_ps[dpc], vv[g][:, sl, dpc * P:(dpc + 1) * P],
                                     pt_exp, start=(scc == 0), stop=(scc == SC - 1))

            # per-row reciprocal as per-partition columns: transpose the raw
            # row sums first, take the reciprocal across 128 lanes (a [1,512]
            # reciprocal on one partition costs ~3.3us and blocks the DVE FIFO)
            nc.vector.tensor_copy(out=rstage[0:1, :], in_=rs_ps)
            rt_ps = ps_small.tile([P, IB], f32, tag="ps_small", name=f"rt_{ib}")
            for ic in range(DC):
                nc.tensor.transpose(rt_ps[:, ic * P:(ic + 1) * P],
                                    rstage[:, ic * P:(ic + 1) * P], ident_f)
            rsum_col = smallp.tile([P, DC], f32, tag="rsum_col", name=f"rsc_{ib}")
            nc.scalar.activation(rsum_col,
                                 rt_ps.rearrange("p (c q) -> p c q", c=DC)[:, :, 0],
                                 AF.Copy)
            rc_sb = smallp.tile([P, DC], f32, tag="rc_sb", name=f"rc_{ib}")
            nc.vector.reciprocal(rc_sb, rsum_col)

            # unnormalized context -> bf16 (no dependency on the reciprocal)
            ctxT = ctxp.tile([P, DC, IB], bf16, tag="ctxT", name=f"ctxT_{ib}")
            for dpc in range(DC):
                nc.vector.tensor_copy(out=ctxT[:, dpc, :], in_=ctx_ps[dpc])

            for ic in range(DC):
                op_ps = ps_acc.tile([P, D], f32, tag="ps_acc", name=f"op_{ib}_{ic}")
                for dpc in range(DC):
                    nc.tensor.matmul(op_ps, ctxT[:, dpc, ic * P:(ic + 1) * P],
                                     w_bf["wo"][:, dpc, :],
                                     start=(dpc == 0), stop=(dpc == DC - 1))
                # scale rows by 1/rowsum on ACT, then add bias on DVE
                ot_s = outp.tile([P, D], f32, tag="out_s", name=f"ots_{ib}_{ic}")
                nc.scalar.activation(ot_s, op_ps, AF.Copy,
                                     scale=rc_sb[:, ic:ic + 1])
                ot = outp.tile([P, D], f32, tag="out_t", name=f"ot_{ib}_{ic}")
                nc.vector.tensor_tensor(ot, ot_s, b_rep["bo"], OP.add)
                nc.sync.dma_start(out_dram[ib * IB + ic * P: ib * IB + (ic + 1) * P, :], ot)

    nc.compile()
    return nc


def _get_nc():
    if "nc" not in _CACHED:
        _CACHED["nc"] = _build_program()
    return _CACHED["nc"]


def _make_in_maps(query, target, wq, bq, wk, bk, wv, bv, wo, bo):
    query = np.asarray(query, dtype=np.float32)
    target = np.asarray(target, dtype=np.float32)
    consts = {
        "wq": np.asarray(wq, np.float32), "bq": np.asarray(bq, np.float32),
        "wk": np.asarray(wk, np.float32), "bk": np.asarray(bk, np.float32),
        "wv": np.asarray(wv, np.float32), "bv": np.asarray(bv, np.float32),
        "wo": np.asarray(wo, np.float32), "bo": np.asarray(bo, np.float32),
    }
    in_maps = []
    for core in range(8):
        b, h = divmod(core, 2)
        in_maps.append({
            "query": np.ascontiguousarray(query[b, h * LQH:(h + 1) * LQH]),
            # faithful to the torch reshape: raw reinterpret of [512, 4096]
            "target": np.ascontiguousarray(target[b]).reshape(S, D),
            **consts,
        })
    return in_maps


def kernel(query, target, wq, bq, wk, bk, wv, bv, wo, bo):
    from concourse import bass_utils
    nc = _get_nc()
    in_maps = _make_in_maps(query, target, wq, bq, wk, bk, wv, bv, wo, bo)
    res = bass_utils.run_bass_kernel_spmd(nc, in_maps, core_ids=list(range(8)))
    out = np.empty((B, LQ, D), np.float32)
    for core in range(8):
        b, h = divmod(core, 2)
        out[b, h * LQH:(h + 1) * LQH] = res.results[core]["out"]
    return out


# revision 12
# speedup vs baseline: 1.3424x; 1.1210x over previous
"""Trainium2 Bass kernel for nn_CrossAttention (B=4, LQ=4096, S=4096, D=512).

Sharding: data-parallel over (batch, query-half): core = 2*b + half.
Each core computes cross-attention for one batch element and 2048 query rows.
K/V projections are recomputed by both cores of a pair (cheap relative to
the attention matmuls).

All heavy matmuls run in bf16 with fp32 PSUM accumulation. Softmax skips the
max-subtraction (scores are bounded ~ +-30 here, exp stays in fp32 range).
The softmax normalization is applied at the very end: the context and output
projection run on unnormalized sums, and the output tiles are scaled by the
per-row reciprocal (obtained as a per-partition column via a PE transpose of
the row-sum reciprocal), which keeps the block tail off the PE critical path.
"""

import numpy as np

B, LQ, S = 4, 4096, 4096
D = 512          # SRC == TGT == 512
P = 128
LQH = LQ // 2    # 2048 query rows per core
DC = D // P      # 4 chunks of the feature dims
SC = S // P      # 32 s-chunks
IB = 512         # block width (query i / kv s) per subtile
NB = LQH // IB   # 4 query blocks
KB = S // IB     # 8 kv blocks

_CACHED = {}


def _build_program():
    import concourse.bass as bass
    import concourse.mybir as mybir
    import concourse.tile as tile
    from concourse import bacc
    from concourse.masks import make_identity
    from contextlib import ExitStack

    f32 = mybir.dt.float32
    bf16 = mybir.dt.bfloat16
    AF = mybir.ActivationFunctionType
    OP = mybir.AluOpType

    nc = bacc.Bacc("TRN2", target_bir_lowering=False, debug=False, num_devices=8)

    query = nc.dram_tensor("query", [LQH, D], f32, kind="ExternalInput").ap()
    target = nc.dram_tensor("target", [S, D], f32, kind="ExternalInput").ap()
    w_dram = {}
    b_dram = {}
    for nm in ("wq", "wk", "wv", "wo"):
        w_dram[nm] = nc.dram_tensor(nm, [D, D], f32, kind="ExternalInput").ap()
    for nm in ("bq", "bk", "bv", "bo"):
        b_dram[nm] = nc.dram_tensor(nm, [D], f32, kind="ExternalInput").ap()
    out_dram = nc.dram_tensor("out", [LQH, D], f32, kind="ExternalOutput").ap()

    with tile.TileContext(nc) as tc, ExitStack() as ctx:
        const = ctx.enter_context(tc.tile_pool(name="const", bufs=1))
        big = ctx.enter_context(tc.tile_pool(name="big", bufs=1))
        wstage = ctx.enter_context(tc.tile_pool(name="wstage", bufs=1))
        ld = ctx.enter_context(tc.tile_pool(name="ld", bufs=6))
        ptp = ctx.enter_context(tc.tile_pool(name="ptp", bufs=6))
        ctxp = ctx.enter_context(tc.tile_pool(name="ctxp", bufs=2))
        outp = ctx.enter_context(tc.tile_pool(name="outp", bufs=2))
        smallp = ctx.enter_context(tc.tile_pool(name="smallp", bufs=2))
        ps_small = ctx.enter_context(tc.tile_pool(name="ps_small", bufs=4, space="PSUM"))
        ps_acc = ctx.enter_context(tc.tile_pool(name="ps_acc", bufs=4, space="PSUM"))

        # ---- constants (tiny, first so nothing waits on them) ----
        ident_b = const.tile([P, P], bf16, tag="ident_b", name="ident_b")
        make_identity(nc, ident_b)
        ident_f = const.tile([P, P], f32, tag="ident_f", name="ident_f")
        make_identity(nc, ident_f)

        ones_col = const.tile([P, 1], f32, tag="ones_col", name="ones_col")
        nc.vector.memset(ones_col, 1.0)

        b_col = {}
        for nm in ("bq", "bk"):
            bc = const.tile([P, DC], f32, tag=f"col_{nm}", name=f"{nm}_col")
            nc.gpsimd.dma_start(out=bc, in_=b_dram[nm].rearrange("(c p) -> p c", p=P))
            b_col[nm] = bc
        b_rep = {}
        for nm in ("bv", "bo"):
            br = const.tile([P, D], f32, tag=f"rep_{nm}", name=f"{nm}_rep")
            src = b_dram[nm]
            bcast = bass.AP(tensor=src.tensor, offset=src.offset,
                            ap=[[0, P]] + list(src.ap))
            nc.gpsimd.dma_start(out=br, in_=bcast)
            b_rep[nm] = br

        # staging row for the reciprocal transpose: row 0 live, rows 1.. zero
        rstage = const.tile([P, IB], f32, tag="rstage", name="rstage")
        nc.vector.memset(rstage, 0.0)

        w_bf = {}

        def load_weight(nm):
            wf = wstage.tile([P, DC, D], f32, tag="w_stage", name=f"{nm}_f32")
            nc.sync.dma_start(wf, w_dram[nm].rearrange("(c p) n -> p c n", p=P))
            wb = const.tile([P, DC, D], bf16, tag=f"w_{nm}", name=f"{nm}_bf")
            nc.vector.tensor_copy(out=wb, in_=wf)
            w_bf[nm] = wb

        def load_transposed(src, row0, dstT):
            # cast 128-row chunks to bf16 and PE-transpose into dstT[:, dc, :]
            for cc in range(IB // P):
                r = row0 + cc * P
                chunk = ld.tile([P, D], f32, tag="in_chunk", name=f"chk_{dstT.name}_{cc}")
                nc.sync.dma_start(chunk, src[r:r + P, :])
                cast = ld.tile([P, D], bf16, tag="in_cast", name=f"cst_{dstT.name}_{cc}")
                nc.vector.tensor_copy(out=cast, in_=chunk)
                psv = ps_acc.tile([P, D], bf16, tag="ps_acc", name=f"pst_{dstT.name}_{cc}")
                for dc in range(DC):
                    nc.tensor.transpose(psv[:, dc * P:(dc + 1) * P],
                                        cast[:, dc * P:(dc + 1) * P], ident_b)
                c = cc * P
                nc.scalar.activation(dstT[:, :, c:c + P],
                                     psv.rearrange("p (c q) -> p c q", c=DC),
                                     AF.Copy)

        # ---- stage A+B (query side) ----
        qinT = [big.tile([P, DC, IB], bf16, tag=f"qinT{i}", name=f"qinT{i}")
                for i in range(NB)]
        qT = [big.tile([P, DC, IB], bf16, tag=f"qT{i}", name=f"qT{i}")
              for i in range(NB)]
        for ibk in range(NB):
            load_transposed(query, ibk * IB, qinT[ibk])
            if ibk == 0:
                # after the first chunk DMAs so transposes start immediately
                load_weight("wq")
            for tcc in range(DC):
                ps = ps_small.tile([P, IB], f32, tag="ps_small", name=f"psq_{tcc}_{ibk}")
                for dc in range(DC):
                    nc.tensor.matmul(ps, w_bf["wq"][:, dc, tcc * P:(tcc + 1) * P],
                                     qinT[ibk][:, dc, :],
                                     start=(dc == 0), stop=(dc == DC - 1))
                nc.vector.tensor_tensor(qT[ibk][:, tcc, :], ps,
                                        b_col["bq"][:, tcc:tcc + 1].to_broadcast([P, IB]),
                                        OP.add)

        # ---- stage A+B (target side) ----
        load_weight("wk")
        load_weight("wv")
        tgtT = [big.tile([P, DC, IB], bf16, tag=f"tgtT{i}", name=f"tgtT{i}")
                for i in range(KB)]
        kT = [big.tile([P, DC, IB], bf16, tag=f"kT{i}", name=f"kT{i}")
              for i in range(KB)]
        vv = [big.tile([P, IB // P, D], bf16, tag=f"vv{i}", name=f"vv{i}")
              for i in range(KB)]

        for sb in range(KB):
            load_transposed(target, sb * IB, tgtT[sb])
            for tcc in range(DC):
                ps = ps_small.tile([P, IB], f32, tag="ps_small", name=f"psk_{tcc}_{sb}")
                for dc in range(DC):
                    nc.tensor.matmul(ps, w_bf["wk"][:, dc, tcc * P:(tcc + 1) * P],
                                     tgtT[sb][:, dc, :],
                                     start=(dc == 0), stop=(dc == DC - 1))
                nc.vector.tensor_tensor(kT[sb][:, tcc, :], ps,
                                        b_col["bk"][:, tcc:tcc + 1].to_broadcast([P, IB]),
                                        OP.add)
            for sl in range(IB // P):
                ps = ps_small.tile([P, D], f32, tag="ps_small", name=f"psv_{sb}_{sl}")
                for dc in range(DC):
                    nc.tensor.matmul(ps, tgtT[sb][:, dc, sl * P:(sl + 1) * P],
                                     w_bf["wv"][:, dc, :],
                                     start=(dc == 0), stop=(dc == DC - 1))
                nc.vector.tensor_tensor(vv[sb][:, sl, :], ps, b_rep["bv"], OP.add)

        load_weight("wo")

        # ---- stage C: attention + output projection, per 512-wide i block ----
        for ib in range(NB):
            ctx_ps = [ps_acc.tile([P, IB], f32, tag="ps_acc", name=f"ctx_{ib}_{d}")
                      for d in range(DC)]
            # partial row sums accumulate on the (otherwise idle) DVE so the
            # PE spends no matmuls on the softmax denominator
            acc = smallp.tile([P, IB], f32, tag="rs_acc", name=f"rsacc_{ib}")

            for scc in range(SC):
                g, sl = divmod(scc, IB // P)
                pt_ps = ps_small.tile([P, IB], f32, tag="ps_small", name=f"pt_{ib}_{scc}")
                for tcc in range(DC):
                    nc.tensor.matmul(pt_ps, kT[g][:, tcc, sl * P:(sl + 1) * P],
                                     qT[ib][:, tcc, :],
                                     start=(tcc == 0), stop=(tcc == DC - 1))
                pt_exp = ptp.tile([P, IB], bf16, tag="pt_exp", name=f"pte_{ib}_{scc}")
                nc.scalar.activation(pt_exp, pt_ps, AF.Exp)
                if scc == 0:
                    nc.vector.tensor_copy(out=acc, in_=pt_exp)
                else:
                    nc.vector.tensor_tensor(acc, acc, pt_exp, OP.add)
                for dpc in range(DC):
                    nc.tensor.matmul(ctx_ps[dpc], vv[g][:, sl, dpc * P:(dpc + 1) * P],
                                     pt_exp, start=(scc == 0), stop=(scc == SC - 1))

            # collapse the partition dim of the partial sums with one tiny
            # M=1 fp32 matmul, then build per-partition reciprocal columns:
            # transpose the row sums first and take the reciprocal across 128
            # lanes (a [1,512] reciprocal on one partition costs ~3.3us and
            # blocks the DVE FIFO)
            rs_ps = ps_small.tile([1, IB], f32, tag="ps_small", name=f"rs_{ib}")
            nc.tensor.matmul(rs_ps, ones_col, acc, start=True, stop=True)
            nc.vector.tensor_copy(out=rstage[0:1, :], in_=rs_ps)
            rt_ps = ps_small.tile([P, IB], f32, tag="ps_small", name=f"rt_{ib}")
            for ic in range(DC):
                nc.tensor.transpose(rt_ps[:, ic * P:(ic + 1) * P],
                                    rstage[:, ic * P:(ic + 1) * P], ident_f)
            rsum_col = smallp.tile([P, DC], f32, tag="rsum_col", name=f"rsc_{ib}")
            nc.scalar.activation(rsum_col,
                                 rt_ps.rearrange("p (c q) -> p c q", c=DC)[:, :, 0],
                                 AF.Copy)
            rc_sb = smallp.tile([P, DC], f32, tag="rc_sb", name=f"rc_{ib}")
            nc.vector.reciprocal(rc_sb, rsum_col)

            # unnormalized context -> bf16 (no dependency on the reciprocal)
            ctxT = ctxp.tile([P, DC, IB], bf16, tag="ctxT", name=f"ctxT_{ib}")
            for dpc in range(DC):
                nc.vector.tensor_copy(out=ctxT[:, dpc, :], in_=ctx_ps[dpc])

            for ic in range(DC):
                op_ps = ps_acc.tile([P, D], f32, tag="ps_acc", name=f"op_{ib}_{ic}")
                for dpc in range(DC):
                    nc.tensor.matmul(op_ps, ctxT[:, dpc, ic * P:(ic + 1) * P],
                                     w_bf["wo"][:, dpc, :],
                                     start=(dpc == 0), stop=(dpc == DC - 1))
                # scale rows by 1/rowsum on ACT, then add bias on DVE
                ot_s = outp.tile([P, D], f32, tag="out_s", name=f"ots_{ib}_{ic}")
                nc.scalar.activation(ot_s, op_ps, AF.Copy,
                                     scale=rc_sb[:, ic:ic + 1])
                ot = outp.tile([P, D], f32, tag="out_t", name=f"ot_{ib}_{ic}")
                nc.vector.tensor_tensor(ot, ot_s, b_rep["bo"], OP.add)
                nc.sync.dma_start(out_dram[ib * IB + ic * P: ib * IB + (ic + 1) * P, :], ot)

    nc.compile()
    return nc


def _get_nc():
    if "nc" not in _CACHED:
        _CACHED["nc"] = _build_program()
    return _CACHED["nc"]


def _make_in_maps(query, target, wq, bq, wk, bk, wv, bv, wo, bo):
    query = np.asarray(query, dtype=np.float32)
    target = np.asarray(target, dtype=np.float32)
    consts = {
        "wq": np.asarray(wq, np.float32), "bq": np.asarray(bq, np.float32),
        "wk": np.asarray(wk, np.float32), "bk": np.asarray(bk, np.float32),
        "wv": np.asarray(wv, np.float32), "bv": np.asarray(bv, np.float32),
        "wo": np.asarray(wo, np.float32), "bo": np.asarray(bo, np.float32),
    }
    in_maps = []
    for core in range(8):
        b, h = divmod(core, 2)
        in_maps.append({
            "query": np.ascontiguousarray(query[b, h * LQH:(h + 1) * LQH]),
            # faithful to the torch reshape: raw reinterpret of [512, 4096]
            "target": np.ascontiguousarray(target[b]).reshape(S, D),
            **consts,
        })
    return in_maps


def kernel(query, target, wq, bq, wk, bk, wv, bv, wo, bo):
    from concourse import bass_utils
    nc = _get_nc()
    in_maps = _make_in_maps(query, target, wq, bq, wk, bk, wv, bv, wo, bo)
    res = bass_utils.run_bass_kernel_spmd(nc, in_maps, core_ids=list(range(8)))
    out = np.empty((B, LQ, D), np.float32)
    for core in range(8):
        b, h = divmod(core, 2)
        out[b, h * LQH:(h + 1) * LQH] = res.results[core]["out"]
    return out
